# revision 23
# baseline (speedup 1.0000x reference)
"""MHSA Trainium2 kernel: B=4, S=2048, D=1024, H=16 heads of 64.

Sharding (8 cores): core c -> batch b=c//2, head-group g=c%2 (8 heads = 512
proj cols). Identical program on every core; only the data differs.

Per-core layouts (host pre-transposes; bf16 matmul operands, f32 PSUM):
  xT  [1024, 2048] = x[b].T          wqT/wkT/wvT [1024, 512] = W[cols].T
  woT [512, 1024]  = Wo[:, cols].T   out [2048, 1024] partial (host sums pairs)

Device program:
  kT = wkT.T @ xT + bk  (all S); qT = wqT.T @ xT + bq (queries 0:1024 only;
    queries 1024:2048 deferred into the sc2=0 attention loop as PE filler,
    1/8 score scale pre-folded into wqT/bq on host)
  v  = xT.T @ wvT       (v bias deferred: bv @ woT added on host)
  per head h: sT = kT[h].T @ qT[h]; P = exp(sT)  [keys on partitions]
    PV with ones-augmented V: out[65, s] = [V_h | 1].T @ P  -> row 64 = sums
    software-pipelined: PV(kt-1) emitted after scores(kt) so exp overlaps
  per head-pair jt: rrec = 1/sums; partition_broadcast -> rec; attnT *= rec
  out_partial = attnT.T @ woT  (spread as 1-matmul filler units in sc2=1)
"""

import os
from contextlib import ExitStack

import numpy as np

import concourse.bass as bass
import concourse.mybir as mybir


def _install_ntff_shim():
    """The agent image's `antenv` lacks `axon_hooks`, which
    run_bass_kernel_spmd imports when trace=True under axon. Provide it,
    wired to the ctypes NTFF hook from trn_agent_boot when available."""
    import sys
    import types
    try:
        from antenv import axon_hooks  # noqa: F401
        return
    except ImportError:
        pass
    try:
        mod = types.ModuleType("antenv.axon_hooks")
        mod._hook = None
        mod.set_axon_ntff_profile_hook = lambda h: setattr(mod, "_hook", h)
        mod.get_axon_ntff_profile_hook = lambda: mod._hook
        import antenv
        sys.modules["antenv.axon_hooks"] = mod
        antenv.axon_hooks = mod
        try:
            from trn_agent_boot.trn_boot import _ntff_profile_via_ctypes
            import os.path
            so = "/opt/axon/libaxon_pjrt.so"
            if os.path.exists(so):
                mod._hook = _ntff_profile_via_ctypes(so)
        except Exception:
            pass
    except Exception:
        pass


_install_ntff_shim()
import concourse.tile as tile
from concourse import bacc
from concourse.bass_utils import run_bass_kernel_spmd

F32 = mybir.dt.float32
F32R = mybir.dt.float32r
BF16 = mybir.dt.bfloat16

S = 2048       # sequence (rows per core's batch)
DF = 1024      # full model dim (contraction for projections)
J = 512        # proj cols per core (8 heads x 64)
HEADS = 8
HD = 64
N_CORES = 8

LAST_RESULT = {}


def _build(mm_dt):
    MDT = mm_dt
    nc = bacc.Bacc(None, target_bir_lowering=False, debug=False)

    xT_d = nc.declare_dram_parameter("xT", [DF, S], MDT, False)
    wqT_d = nc.declare_dram_parameter("wqT", [DF, J], MDT, False)
    wkT_d = nc.declare_dram_parameter("wkT", [DF, J], MDT, False)
    wvT_d = nc.declare_dram_parameter("wvT", [DF, J], MDT, False)
    bq_d = nc.declare_dram_parameter("bq", [J], F32, False)
    bk_d = nc.declare_dram_parameter("bk", [J], F32, False)
    woT_d = nc.declare_dram_parameter("woT", [J, DF], MDT, False)
    ones_d = nc.declare_dram_parameter("ones", [128, HEADS], MDT, False)
    out_d = nc.declare_dram_parameter("out", [S, DF], F32, isOutput=True)

    with tile.TileContext(nc) as tc, ExitStack() as ctx:
        persist = ctx.enter_context(tc.tile_pool(name="persist", bufs=1))
        wkvp = ctx.enter_context(tc.tile_pool(name="wkv", bufs=1))
        xs = ctx.enter_context(tc.tile_pool(name="xs", bufs=1))
        ptp = ctx.enter_context(tc.tile_pool(name="pt", bufs=6))
        osbp = ctx.enter_context(tc.tile_pool(name="osb", bufs=4))
        recp = ctx.enter_context(tc.tile_pool(name="rec", bufs=3))
        zrecp = ctx.enter_context(tc.tile_pool(name="zrec", bufs=3))
        stagp = ctx.enter_context(tc.tile_pool(name="stag", bufs=3))
        zdram = ctx.enter_context(tc.tile_pool(name="zdram", bufs=3, space="DRAM"))
        aux = ctx.enter_context(tc.tile_pool(name="aux", bufs=2, space="PSUM"))
        psS = ctx.enter_context(tc.tile_pool(name="psS", bufs=4, space="PSUM"))
        psPV = ctx.enter_context(tc.tile_pool(name="psPV", bufs=1, space="PSUM"))

        qT = [persist.tile([128, S], MDT, name=f"qT{i}", tag=f"qT{i}") for i in range(4)]
        kT = [persist.tile([128, S], MDT, name=f"kT{i}", tag=f"kT{i}") for i in range(4)]
        vt = [persist.tile([128, HEADS, HD + 1], MDT, name=f"v{i}", tag=f"v{i}")
              for i in range(16)]
        attnT = [persist.tile([128, S], MDT, name=f"at{i}", tag=f"at{i}")
                 for i in range(4)]
        wq_sb = [persist.tile([128, J], MDT, name=f"wq{k}", tag=f"wq{k}")
                 for k in range(8)]
        wo_sb = [persist.tile([128, DF], MDT, name=f"wo{i}", tag=f"wo{i}")
                 for i in range(4)]
        bq_sb = persist.tile([128, 4], F32, name="bq", tag="bq")
        bk_sb = persist.tile([128, 4], F32, name="bk", tag="bk")


        nc.sync.dma_start(out=bq_sb, in_=bq_d[:].rearrange("(a p) -> p a", p=128))
        nc.sync.dma_start(out=bk_sb, in_=bk_d[:].rearrange("(a p) -> p a", p=128))

        # startup burst: spread the first-chain inputs (wk + x0) over four
        # idle engine DMA rings so the PE can start ~5us in.
        rings = [nc.sync, nc.gpsimd, nc.scalar]
        wk_sb = [wkvp.tile([128, J], MDT, name=f"wk{k}", tag=f"wkv{k}")
                 for k in range(8)]

        def load_x(sc, spread=False):
            ts = []
            for kc in range(8):
                t = xs.tile([128, 512], MDT, name=f"xt{sc % 2}_{kc}",
                            tag=f"xt{sc % 2}_{kc}")
                eng = rings[kc % 3] if spread else nc.sync
                eng.dma_start(
                    out=t,
                    in_=xT_d[128 * kc:128 * (kc + 1), 512 * sc:512 * (sc + 1)])
                ts.append(t)
            return ts

        for k in range(8):
            rings[k % 3].dma_start(out=wk_sb[k], in_=wkT_d[128 * k:128 * (k + 1), :])
        xt_by_sc = {0: load_x(0, spread=True)}
        for k in range(8):
            nc.gpsimd.dma_start(out=wq_sb[k], in_=wqT_d[128 * k:128 * (k + 1), :])
        for i in range(4):
            nc.gpsimd.dma_start(out=wo_sb[i], in_=woT_d[128 * i:128 * (i + 1), :])

        # ---- Phase A: kT (all S), qT (queries 0:1024), then v pass.
        for sc in range(4):
            ss = slice(512 * sc, 512 * (sc + 1))
            xt = xt_by_sc.get(sc) or load_x(sc)
            xt_by_sc[sc] = xt
            for jt in range(4):
                jj = slice(128 * jt, 128 * (jt + 1))
                ps = aux.tile([128, 512], F32, name="aux", tag="aux")
                for kc in range(8):
                    nc.tensor.matmul(ps, wk_sb[kc][:, jj], xt[kc],
                                     start=(kc == 0), stop=(kc == 7))
                nc.vector.tensor_scalar_add(kT[jt][:, ss], ps, bk_sb[:, jt:jt + 1])
                if sc < 2:
                    ps = aux.tile([128, 512], F32, name="aux", tag="aux")
                    for kc in range(8):
                        nc.tensor.matmul(ps, wq_sb[kc][:, jj], xt[kc],
                                         start=(kc == 0), stop=(kc == 7))
                    nc.vector.tensor_scalar_add(qT[jt][:, ss], ps, bq_sb[:, jt:jt + 1])
        # v pass: wv overwrites wk slots (same tags -> WAR-synced)
        wv_sb = [wkvp.tile([128, J], MDT, name=f"wv{k}", tag=f"wkv{k}")
                 for k in range(8)]
        for k in range(8):
            nc.gpsimd.dma_start(out=wv_sb[k], in_=wvT_d[128 * k:128 * (k + 1), :])
        for sc in range(4):
            xt = load_x(sc)
            xt_by_sc[sc] = xt
            for stl in range(4):
                st = 4 * sc + stl
                ps = aux.tile([128, 512], F32, name="aux", tag="aux")
                for kc in range(8):
                    nc.tensor.matmul(ps, xt[kc][:, 128 * stl:128 * (stl + 1)],
                                     wv_sb[kc], start=(kc == 0), stop=(kc == 7))
                nc.vector.tensor_copy(
                    vt[st][:, :, 0:HD], ps[:].rearrange("p (h d) -> p h d", h=HEADS))
                nc.sync.dma_start(
                    out=vt[st][:, :, HD:HD + 1],
                    in_=ones_d[:].rearrange("p (a b) -> p a b", b=1))

        # ---- Phase B: attention, software-pipelined (pv lags scores by 1 kt),
        # with one filler matmul every other kt:
        #   sc2=0 fillers: deferred q-proj chains (queries 1024:2048), reading
        #     the x chunks still resident in the xs slots from the v pass.
        #   sc2=1 fillers: out-proj of sc2=0's rows, one matmul at a time.
        def qfill_units():
            for sc in (2, 3):
                xt = xt_by_sc[sc]
                ss = slice(512 * sc, 512 * (sc + 1))
                for jt in range(4):
                    jj = slice(128 * jt, 128 * (jt + 1))
                    ps = None
                    for kc in range(8):
                        if kc == 0:
                            ps = aux.tile([128, 512], F32, name="aux", tag="aux")
                        nc.tensor.matmul(ps, wq_sb[kc][:, jj], xt[kc],
                                         start=(kc == 0), stop=(kc == 7))
                        if kc == 7:
                            nc.vector.tensor_scalar_add(
                                qT[jt][:, ss], ps, bq_sb[:, jt:jt + 1])
                        yield

        def emit_oproj_group(st, oc):
            sl = slice(128 * st, 128 * (st + 1))
            ocs = slice(512 * oc, 512 * (oc + 1))
            ps = None
            for jc in range(4):
                if jc == 0:
                    ps = aux.tile([128, 512], F32, name="aux", tag="aux")
                nc.tensor.matmul(ps, attnT[jc][:, sl], wo_sb[jc][:, ocs],
                                 start=(jc == 0), stop=(jc == 3))
                if jc == 3:
                    o_sb = osbp.tile([128, 512], F32, name="osb", tag="osb")
                    nc.vector.tensor_copy(o_sb, ps)
                    nc.gpsimd.dma_start(out=out_d[sl, ocs], in_=o_sb)
                yield

        def oproj_units(sts):
            for st in sts:
                for oc in range(2):
                    yield from emit_oproj_group(st, oc)

        for sc2 in range(2):
            s0 = 1024 * sc2
            filler = qfill_units() if sc2 == 0 else oproj_units(range(8))
            for h in range(HEADS):
                jt, ro = h // 2, 64 * (h % 2)
                rows = slice(ro, ro + 64)
                pv_ps = psPV.tile([65, 1024], F32, name="pv", tag="pv")
                pts = {}

                def emit_pv(k):
                    pa, pb = pts.pop(k)
                    nc.tensor.matmul(pv_ps[:, 0:512], vt[k][:, h, :], pa,
                                     start=(k == 0), stop=(k == 15))
                    nc.tensor.matmul(pv_ps[:, 512:1024], vt[k][:, h, :], pb,
                                     start=(k == 0), stop=(k == 15))

                # Scores/exp split in 512-wide halves: each exp depends on a
                # single matmul, and psS (4x single-bank tiles) gives the
                # score matmuls ~2 kt of WAR slack so the scalar never
                # bubbles. PV lags scores by TWO kt so the in-order PE never
                # blocks on the scalar either.
                for kt in range(16):
                    tt = slice(128 * kt, 128 * (kt + 1))
                    sa = psS.tile([128, 512], F32, name="sps", tag="sps")
                    nc.tensor.matmul(sa, kT[jt][rows, tt],
                                     qT[jt][rows, s0:s0 + 512])
                    pa = ptp.tile([128, 512], MDT, name="pt", tag="pt")
                    nc.scalar.activation(pa, sa, mybir.ActivationFunctionType.Exp)
                    sb = psS.tile([128, 512], F32, name="sps", tag="sps")
                    nc.tensor.matmul(sb, kT[jt][rows, tt],
                                     qT[jt][rows, s0 + 512:s0 + 1024])
                    pb = ptp.tile([128, 512], MDT, name="pt", tag="pt")
                    nc.scalar.activation(pb, sb, mybir.ActivationFunctionType.Exp)
                    pts[kt] = (pa, pb)
                    if kt >= 2:
                        emit_pv(kt - 2)
                    if kt in (4, 8, 12):
                        next(filler, None)
                emit_pv(14)
                emit_pv(15)
                # evict: 1/Z straight off the PSUM sums row, then the 64 attn
                # rows (both 32-aligned partition counts — a merged 65-row op
                # hits a ~6x DVE slow path); 5 boundary fillers + next head's
                # scores cover the ~2.4us DVE bank-hold on the PE.
                zrec = zrecp.tile([1, 1024], MDT, name="zrec", tag="zrec")
                with nc.allow_low_precision(reason="bf16 1/Z, tol 2e-2"):
                    nc.vector.reciprocal(zrec, pv_ps[64:65, :])
                stag = stagp.tile([64, 1024], MDT, name="stag", tag="stag")
                nc.vector.tensor_copy(stag, pv_ps[0:64, :])
                for _ in range(5):
                    next(filler, None)
                zd = zdram.tile([1, 1024], MDT, name="zd", tag="zd")
                nc.gpsimd.dma_start(out=zd, in_=zrec)
                rec = recp.tile([64, 1024], MDT, name="rec", tag="rec")
                nc.gpsimd.dma_start(out=rec,
                                    in_=zd[0:1, :].partition_broadcast(64))
                nc.vector.tensor_mul(attnT[jt][ro:ro + 64, s0:s0 + 1024],
                                     stag, rec)
            for _ in filler:
                pass
        for _ in oproj_units(range(8, 16)):
            pass
    nc.compile()
    return nc


_NC_CACHE = {}


def _get_nc(mm_dt):
    key = str(mm_dt)
    if key not in _NC_CACHE:
        _NC_CACHE[key] = _build(mm_dt)
    return _NC_CACHE[key]


def kernel(**inputs):
    x = np.asarray(inputs["x"], np.float32)
    Wq = np.asarray(inputs["Wq"], np.float32)
    bq = np.asarray(inputs["bq"], np.float32)
    Wk = np.asarray(inputs["Wk"], np.float32)
    bk = np.asarray(inputs["bk"], np.float32)
    Wv = np.asarray(inputs["Wv"], np.float32)
    bv = np.asarray(inputs["bv"], np.float32)
    Wo = np.asarray(inputs["Wo"], np.float32)
    bo = np.asarray(inputs["bo"], np.float32)

    scale = np.float32(1.0 / np.sqrt(HD))
    mm_dt = {"f32r": F32R, "bf16": BF16}[os.environ.get("BASS_MM_DT", "bf16")]
    if mm_dt is BF16:
        import ml_dtypes
        host_dt = ml_dtypes.bfloat16
    else:
        host_dt = np.float32
    nc = _get_nc(mm_dt)

    in_maps = []
    bvwo = []     # host-side bv @ woT rows, one per core
    for c in range(N_CORES):
        b, g = c // 2, c % 2
        cols = slice(J * g, J * (g + 1))
        woTs = np.ascontiguousarray(Wo[:, cols].T)
        in_maps.append({
            "xT": np.ascontiguousarray(x[b].T).astype(host_dt),
            "wqT": (np.ascontiguousarray(Wq[cols, :].T) * scale).astype(host_dt),
            "wkT": np.ascontiguousarray(Wk[cols, :].T).astype(host_dt),
            "wvT": np.ascontiguousarray(Wv[cols, :].T).astype(host_dt),
            "bq": np.ascontiguousarray(bq[cols]) * scale,
            "bk": np.ascontiguousarray(bk[cols]),
            "woT": woTs.astype(host_dt),
            "ones": np.ones((128, HEADS), host_dt),
        })
        bvwo.append(bv[cols] @ woTs)

    res = run_bass_kernel_spmd(
        nc, in_maps, list(range(N_CORES)),
        trace=bool(os.environ.get("BASS_TRACE")))
    LAST_RESULT["exec_time_ns"] = res.exec_time_ns
    LAST_RESULT["mean_exec_time_ns"] = getattr(res, "mean_exec_time_ns", None)
    LAST_RESULT["profile_json"] = res.profile_json
    it = res.instructions_and_trace
    LAST_RESULT["trace_path"] = it[1] if it else None
    LAST_RESULT["insts"] = it[0] if it else None

    B = x.shape[0]
    out = np.empty((B, S, DF), np.float32)
    for b in range(B):
        out[b] = (res.results[2 * b]["out"] + res.results[2 * b + 1]["out"]
                  + bvwo[2 * b][None, :] + bvwo[2 * b + 1][None, :]
                  + bo[None, :])
    return out


# revision 26
# speedup vs baseline: 1.0397x; 1.0397x over previous
"""MHSA Trainium2 kernel: B=4, S=2048, D=1024, H=16 heads of 64.

Sharding (8 cores): core c -> batch b=c//2, head-group g=c%2 (8 heads = 512
proj cols). Identical program on every core; only the data differs.

Per-core layouts (host pre-transposes; bf16 matmul operands, f32 PSUM):
  xT  [1024, 2048] = x[b].T          wqT/wkT/wvT [1024, 512] = W[cols].T
  woT [512, 1024]  = Wo[:, cols].T   out [2048, 1024] partial (host sums pairs)

Device program:
  kT = wkT.T @ xT + bk  (all S); qT = wqT.T @ xT + bq (queries 0:1024 only;
    queries 1024:2048 deferred into the sc2=0 attention loop as PE filler,
    1/8 score scale pre-folded into wqT/bq on host)
  v  = xT.T @ wvT       (v bias deferred: bv @ woT added on host)
  per head h: sT = kT[h].T @ qT[h]; P = exp(sT)  [keys on partitions]
    PV with ones-augmented V: out[65, s] = [V_h | 1].T @ P  -> row 64 = sums
    software-pipelined: PV(kt-1) emitted after scores(kt) so exp overlaps
  per head-pair jt: rrec = 1/sums; partition_broadcast -> rec; attnT *= rec
  out_partial = attnT.T @ woT  (spread as 1-matmul filler units in sc2=1)
"""

import os
from contextlib import ExitStack

import numpy as np

import concourse.bass as bass
import concourse.mybir as mybir


def _install_ntff_shim():
    """The agent image's `antenv` lacks `axon_hooks`, which
    run_bass_kernel_spmd imports when trace=True under axon. Provide it,
    wired to the ctypes NTFF hook from trn_agent_boot when available."""
    import sys
    import types
    try:
        from antenv import axon_hooks  # noqa: F401
        return
    except ImportError:
        pass
    try:
        mod = types.ModuleType("antenv.axon_hooks")
        mod._hook = None
        mod.set_axon_ntff_profile_hook = lambda h: setattr(mod, "_hook", h)
        mod.get_axon_ntff_profile_hook = lambda: mod._hook
        import antenv
        sys.modules["antenv.axon_hooks"] = mod
        antenv.axon_hooks = mod
        try:
            from trn_agent_boot.trn_boot import _ntff_profile_via_ctypes
            import os.path
            so = "/opt/axon/libaxon_pjrt.so"
            if os.path.exists(so):
                mod._hook = _ntff_profile_via_ctypes(so)
        except Exception:
            pass
    except Exception:
        pass


_install_ntff_shim()
import concourse.tile as tile
from concourse import bacc
from concourse.bass_utils import run_bass_kernel_spmd

F32 = mybir.dt.float32
F32R = mybir.dt.float32r
BF16 = mybir.dt.bfloat16

S = 2048       # sequence (rows per core's batch)
DF = 1024      # full model dim (contraction for projections)
J = 512        # proj cols per core (8 heads x 64)
HEADS = 8
HD = 64
N_CORES = 8

LAST_RESULT = {}


def _build(mm_dt):
    MDT = mm_dt
    nc = bacc.Bacc(None, target_bir_lowering=False, debug=False)

    xT_d = nc.declare_dram_parameter("xT", [DF, S], MDT, False)
    wqT_d = nc.declare_dram_parameter("wqT", [DF, J], MDT, False)
    wkT_d = nc.declare_dram_parameter("wkT", [DF, J], MDT, False)
    wvT_d = nc.declare_dram_parameter("wvT", [DF, J], MDT, False)
    bq_d = nc.declare_dram_parameter("bq", [J], F32, False)
    bk_d = nc.declare_dram_parameter("bk", [J], F32, False)
    woT_d = nc.declare_dram_parameter("woT", [J, DF], MDT, False)
    ones_d = nc.declare_dram_parameter("ones", [128, HEADS], MDT, False)
    out_d = nc.declare_dram_parameter("out", [S, DF], F32, isOutput=True)

    with tile.TileContext(nc) as tc, ExitStack() as ctx:
        persist = ctx.enter_context(tc.tile_pool(name="persist", bufs=1))
        wkvp = ctx.enter_context(tc.tile_pool(name="wkv", bufs=1))
        xs = ctx.enter_context(tc.tile_pool(name="xs", bufs=1))
        ptp = ctx.enter_context(tc.tile_pool(name="pt", bufs=4))
        osbp = ctx.enter_context(tc.tile_pool(name="osb", bufs=4))
        recp = ctx.enter_context(tc.tile_pool(name="rec", bufs=3))
        zrecp = ctx.enter_context(tc.tile_pool(name="zrec", bufs=3))
        stagp = ctx.enter_context(tc.tile_pool(name="stag", bufs=3))
        zdram = ctx.enter_context(tc.tile_pool(name="zdram", bufs=3, space="DRAM"))
        aux = ctx.enter_context(tc.tile_pool(name="aux", bufs=2, space="PSUM"))
        psS = ctx.enter_context(tc.tile_pool(name="psS", bufs=2, space="PSUM"))
        psPV = ctx.enter_context(tc.tile_pool(name="psPV", bufs=1, space="PSUM"))

        qT = [persist.tile([128, S], MDT, name=f"qT{i}", tag=f"qT{i}") for i in range(4)]
        kT = [persist.tile([128, S], MDT, name=f"kT{i}", tag=f"kT{i}") for i in range(4)]
        vt = [persist.tile([128, HEADS, HD + 1], MDT, name=f"v{i}", tag=f"v{i}")
              for i in range(16)]
        attnT = [persist.tile([128, S], MDT, name=f"at{i}", tag=f"at{i}")
                 for i in range(4)]
        wq_sb = [persist.tile([128, J], MDT, name=f"wq{k}", tag=f"wq{k}")
                 for k in range(8)]
        wo_sb = [persist.tile([128, DF], MDT, name=f"wo{i}", tag=f"wo{i}")
                 for i in range(4)]
        bq_sb = persist.tile([128, 4], F32, name="bq", tag="bq")
        bk_sb = persist.tile([128, 4], F32, name="bk", tag="bk")


        nc.sync.dma_start(out=bq_sb, in_=bq_d[:].rearrange("(a p) -> p a", p=128))
        nc.sync.dma_start(out=bk_sb, in_=bk_d[:].rearrange("(a p) -> p a", p=128))

        # startup burst: spread the first-chain inputs (wk + x0) over four
        # idle engine DMA rings so the PE can start ~5us in.
        rings = [nc.sync, nc.gpsimd, nc.scalar]
        wk_sb = [wkvp.tile([128, J], MDT, name=f"wk{k}", tag=f"wkv{k}")
                 for k in range(8)]

        def load_x(sc, spread=False):
            ts = []
            for kc in range(8):
                t = xs.tile([128, 512], MDT, name=f"xt{sc % 2}_{kc}",
                            tag=f"xt{sc % 2}_{kc}")
                eng = rings[kc % 3] if spread else nc.sync
                eng.dma_start(
                    out=t,
                    in_=xT_d[128 * kc:128 * (kc + 1), 512 * sc:512 * (sc + 1)])
                ts.append(t)
            return ts

        for k in range(8):
            rings[k % 3].dma_start(out=wk_sb[k], in_=wkT_d[128 * k:128 * (k + 1), :])
        xt_by_sc = {0: load_x(0, spread=True)}
        for k in range(8):
            nc.gpsimd.dma_start(out=wq_sb[k], in_=wqT_d[128 * k:128 * (k + 1), :])
        for i in range(4):
            nc.gpsimd.dma_start(out=wo_sb[i], in_=woT_d[128 * i:128 * (i + 1), :])

        # ---- Phase A: kT (all S), qT (queries 0:1024), then v pass.
        for sc in range(4):
            ss = slice(512 * sc, 512 * (sc + 1))
            xt = xt_by_sc.get(sc) or load_x(sc)
            xt_by_sc[sc] = xt
            for jt in range(4):
                jj = slice(128 * jt, 128 * (jt + 1))
                ps = aux.tile([128, 512], F32, name="aux", tag="aux")
                for kc in range(8):
                    nc.tensor.matmul(ps, wk_sb[kc][:, jj], xt[kc],
                                     start=(kc == 0), stop=(kc == 7))
                nc.vector.tensor_scalar_add(kT[jt][:, ss], ps, bk_sb[:, jt:jt + 1])
                if sc < 2:
                    ps = aux.tile([128, 512], F32, name="aux", tag="aux")
                    for kc in range(8):
                        nc.tensor.matmul(ps, wq_sb[kc][:, jj], xt[kc],
                                         start=(kc == 0), stop=(kc == 7))
                    nc.vector.tensor_scalar_add(qT[jt][:, ss], ps, bq_sb[:, jt:jt + 1])
        # v pass: wv overwrites wk slots (same tags -> WAR-synced)
        wv_sb = [wkvp.tile([128, J], MDT, name=f"wv{k}", tag=f"wkv{k}")
                 for k in range(8)]
        for k in range(8):
            nc.gpsimd.dma_start(out=wv_sb[k], in_=wvT_d[128 * k:128 * (k + 1), :])
        for sc in range(4):
            xt = load_x(sc)
            xt_by_sc[sc] = xt
            for stl in range(4):
                st = 4 * sc + stl
                ps = aux.tile([128, 512], F32, name="aux", tag="aux")
                for kc in range(8):
                    nc.tensor.matmul(ps, xt[kc][:, 128 * stl:128 * (stl + 1)],
                                     wv_sb[kc], start=(kc == 0), stop=(kc == 7))
                nc.vector.tensor_copy(
                    vt[st][:, :, 0:HD], ps[:].rearrange("p (h d) -> p h d", h=HEADS))
                nc.sync.dma_start(
                    out=vt[st][:, :, HD:HD + 1],
                    in_=ones_d[:].rearrange("p (a b) -> p a b", b=1))

        # ---- Phase B: attention, software-pipelined (pv lags scores by 1 kt),
        # with one filler matmul every other kt:
        #   sc2=0 fillers: deferred q-proj chains (queries 1024:2048), reading
        #     the x chunks still resident in the xs slots from the v pass.
        #   sc2=1 fillers: out-proj of sc2=0's rows, one matmul at a time.
        def qfill_units():
            for sc in (2, 3):
                xt = xt_by_sc[sc]
                ss = slice(512 * sc, 512 * (sc + 1))
                for jt in range(4):
                    jj = slice(128 * jt, 128 * (jt + 1))
                    ps = None
                    for kc in range(8):
                        if kc == 0:
                            ps = aux.tile([128, 512], F32, name="aux", tag="aux")
                        nc.tensor.matmul(ps, wq_sb[kc][:, jj], xt[kc],
                                         start=(kc == 0), stop=(kc == 7))
                        if kc == 7:
                            nc.vector.tensor_scalar_add(
                                qT[jt][:, ss], ps, bq_sb[:, jt:jt + 1])
                        yield

        def emit_oproj_group(st, oc):
            sl = slice(128 * st, 128 * (st + 1))
            ocs = slice(512 * oc, 512 * (oc + 1))
            ps = None
            for jc in range(4):
                if jc == 0:
                    ps = aux.tile([128, 512], F32, name="aux", tag="aux")
                nc.tensor.matmul(ps, attnT[jc][:, sl], wo_sb[jc][:, ocs],
                                 start=(jc == 0), stop=(jc == 3))
                if jc == 3:
                    o_sb = osbp.tile([128, 512], F32, name="osb", tag="osb")
                    nc.vector.tensor_copy(o_sb, ps)
                    nc.gpsimd.dma_start(out=out_d[sl, ocs], in_=o_sb)
                yield

        def oproj_units(sts):
            for st in sts:
                for oc in range(2):
                    yield from emit_oproj_group(st, oc)

        for sc2 in range(2):
            s0 = 1024 * sc2
            filler = qfill_units() if sc2 == 0 else oproj_units(range(8))
            for h in range(HEADS):
                jt, ro = h // 2, 64 * (h % 2)
                rows = slice(ro, ro + 64)
                pv_ps = psPV.tile([65, 1024], F32, name="pv", tag="pv")
                pts = {}

                def emit_pv(k):
                    pt = pts.pop(k)
                    nc.tensor.matmul(pv_ps[:, 0:512], vt[k][:, h, :],
                                     pt[:, 0:512], start=(k == 0), stop=(k == 15))
                    nc.tensor.matmul(pv_ps[:, 512:1024], vt[k][:, h, :],
                                     pt[:, 512:1024], start=(k == 0), stop=(k == 15))

                # PV lags scores by TWO kt: pv(kt-2)'s exp finished a full
                # period ago, so the in-order PE never blocks on the scalar.
                for kt in range(16):
                    tt = slice(128 * kt, 128 * (kt + 1))
                    s_ps = psS.tile([128, 1024], F32, name="sps", tag="sps")
                    nc.tensor.matmul(s_ps[:, 0:512], kT[jt][rows, tt],
                                     qT[jt][rows, s0:s0 + 512])
                    nc.tensor.matmul(s_ps[:, 512:1024], kT[jt][rows, tt],
                                     qT[jt][rows, s0 + 512:s0 + 1024])
                    pt = ptp.tile([128, 1024], MDT, name="pt", tag="pt")
                    nc.scalar.activation(pt, s_ps, mybir.ActivationFunctionType.Exp)
                    pts[kt] = pt
                    if kt >= 2:
                        emit_pv(kt - 2)
                    if kt in (4, 8, 12):
                        next(filler, None)
                emit_pv(14)
                emit_pv(15)
                # evict: 1/Z straight off the PSUM sums row, then the attn
                # rows as TWO bank-local [64,512] copies — a [64,1024] DVE
                # copy spanning 2 PSUM banks runs ~6x slow (103us/kernel!);
                # 5 boundary fillers + next head's scores cover the ~2.5us
                # DVE bank-hold on the PE.
                zrec = zrecp.tile([1, 1024], MDT, name="zrec", tag="zrec")
                with nc.allow_low_precision(reason="bf16 1/Z, tol 2e-2"):
                    nc.vector.reciprocal(zrec, pv_ps[64:65, :])
                stag = stagp.tile([64, 1024], MDT, name="stag", tag="stag")
                nc.vector.tensor_copy(stag[:, 0:512], pv_ps[0:64, 0:512])
                nc.vector.tensor_copy(stag[:, 512:1024], pv_ps[0:64, 512:1024])
                for _ in range(5):
                    next(filler, None)
                zd = zdram.tile([1, 1024], MDT, name="zd", tag="zd")
                nc.gpsimd.dma_start(out=zd, in_=zrec)
                rec = recp.tile([64, 1024], MDT, name="rec", tag="rec")
                nc.gpsimd.dma_start(out=rec,
                                    in_=zd[0:1, :].partition_broadcast(64))
                nc.vector.tensor_mul(attnT[jt][ro:ro + 64, s0:s0 + 1024],
                                     stag, rec)
            for _ in filler:
                pass
        for _ in oproj_units(range(8, 16)):
            pass
    nc.compile()
    return nc


_NC_CACHE = {}


def _get_nc(mm_dt):
    key = str(mm_dt)
    if key not in _NC_CACHE:
        _NC_CACHE[key] = _build(mm_dt)
    return _NC_CACHE[key]


def kernel(**inputs):
    x = np.asarray(inputs["x"], np.float32)
    Wq = np.asarray(inputs["Wq"], np.float32)
    bq = np.asarray(inputs["bq"], np.float32)
    Wk = np.asarray(inputs["Wk"], np.float32)
    bk = np.asarray(inputs["bk"], np.float32)
    Wv = np.asarray(inputs["Wv"], np.float32)
    bv = np.asarray(inputs["bv"], np.float32)
    Wo = np.asarray(inputs["Wo"], np.float32)
    bo = np.asarray(inputs["bo"], np.float32)

    scale = np.float32(1.0 / np.sqrt(HD))
    mm_dt = {"f32r": F32R, "bf16": BF16}[os.environ.get("BASS_MM_DT", "bf16")]
    if mm_dt is BF16:
        import ml_dtypes
        host_dt = ml_dtypes.bfloat16
    else:
        host_dt = np.float32
    nc = _get_nc(mm_dt)

    in_maps = []
    bvwo = []     # host-side bv @ woT rows, one per core
    for c in range(N_CORES):
        b, g = c // 2, c % 2
        cols = slice(J * g, J * (g + 1))
        woTs = np.ascontiguousarray(Wo[:, cols].T)
        in_maps.append({
            "xT": np.ascontiguousarray(x[b].T).astype(host_dt),
            "wqT": (np.ascontiguousarray(Wq[cols, :].T) * scale).astype(host_dt),
            "wkT": np.ascontiguousarray(Wk[cols, :].T).astype(host_dt),
            "wvT": np.ascontiguousarray(Wv[cols, :].T).astype(host_dt),
            "bq": np.ascontiguousarray(bq[cols]) * scale,
            "bk": np.ascontiguousarray(bk[cols]),
            "woT": woTs.astype(host_dt),
            "ones": np.ones((128, HEADS), host_dt),
        })
        bvwo.append(bv[cols] @ woTs)

    res = run_bass_kernel_spmd(
        nc, in_maps, list(range(N_CORES)),
        trace=bool(os.environ.get("BASS_TRACE")))
    LAST_RESULT["exec_time_ns"] = res.exec_time_ns
    LAST_RESULT["mean_exec_time_ns"] = getattr(res, "mean_exec_time_ns", None)
    LAST_RESULT["profile_json"] = res.profile_json
    it = res.instructions_and_trace
    LAST_RESULT["trace_path"] = it[1] if it else None
    LAST_RESULT["insts"] = it[0] if it else None

    B = x.shape[0]
    out = np.empty((B, S, DF), np.float32)
    for b in range(B):
        out[b] = (res.results[2 * b]["out"] + res.results[2 * b + 1]["out"]
                  + bvwo[2 * b][None, :] + bvwo[2 * b + 1][None, :]
                  + bo[None, :])
    return out


# revision 28
# speedup vs baseline: 1.3710x; 1.3186x over previous
"""MHSA Trainium2 kernel: B=4, S=2048, D=1024, H=16 heads of 64.

Sharding (8 cores): core c -> batch b=c//2, head-group g=c%2 (8 heads = 512
proj cols). Identical program on every core; only the data differs.

Per-core layouts (host pre-transposes; bf16 matmul operands, f32 PSUM):
  xT  [1024, 2048] = x[b].T          wqT/wkT/wvT [1024, 512] = W[cols].T
  woT [512, 1024]  = Wo[:, cols].T   out [2048, 1024] partial (host sums pairs)

Device program:
  kT = wkT.T @ xT + bk  (all S); qT = wqT.T @ xT + bq (queries 0:1024 only;
    queries 1024:2048 deferred into the sc2=0 attention loop as PE filler,
    1/8 score scale pre-folded into wqT/bq on host)
  v  = xT.T @ wvT       (v bias deferred: bv @ woT added on host)
  per head h: sT = kT[h].T @ qT[h]; P = exp(sT)  [keys on partitions]
    PV with ones-augmented V: out[65, s] = [V_h | 1].T @ P  -> row 64 = sums
    software-pipelined: PV(kt-1) emitted after scores(kt) so exp overlaps
  per head-pair jt: rrec = 1/sums; partition_broadcast -> rec; attnT *= rec
  out_partial = attnT.T @ woT  (spread as 1-matmul filler units in sc2=1)
"""

import os
from contextlib import ExitStack

import numpy as np

import concourse.bass as bass
import concourse.mybir as mybir


def _install_ntff_shim():
    """The agent image's `antenv` lacks `axon_hooks`, which
    run_bass_kernel_spmd imports when trace=True under axon. Provide it,
    wired to the ctypes NTFF hook from trn_agent_boot when available."""
    import sys
    import types
    try:
        from antenv import axon_hooks  # noqa: F401
        return
    except ImportError:
        pass
    try:
        mod = types.ModuleType("antenv.axon_hooks")
        mod._hook = None
        mod.set_axon_ntff_profile_hook = lambda h: setattr(mod, "_hook", h)
        mod.get_axon_ntff_profile_hook = lambda: mod._hook
        import antenv
        sys.modules["antenv.axon_hooks"] = mod
        antenv.axon_hooks = mod
        try:
            from trn_agent_boot.trn_boot import _ntff_profile_via_ctypes
            import os.path
            so = "/opt/axon/libaxon_pjrt.so"
            if os.path.exists(so):
                mod._hook = _ntff_profile_via_ctypes(so)
        except Exception:
            pass
    except Exception:
        pass


_install_ntff_shim()
import concourse.tile as tile
from concourse import bacc
from concourse.bass_utils import run_bass_kernel_spmd

F32 = mybir.dt.float32
F32R = mybir.dt.float32r
BF16 = mybir.dt.bfloat16

S = 2048       # sequence (rows per core's batch)
DF = 1024      # full model dim (contraction for projections)
J = 512        # proj cols per core (8 heads x 64)
HEADS = 8
HD = 64
N_CORES = 8

LAST_RESULT = {}


def _build(mm_dt):
    MDT = mm_dt
    nc = bacc.Bacc(None, target_bir_lowering=False, debug=False)

    xT_d = nc.declare_dram_parameter("xT", [DF, S], MDT, False)
    wqT_d = nc.declare_dram_parameter("wqT", [DF, J], MDT, False)
    wkT_d = nc.declare_dram_parameter("wkT", [DF, J], MDT, False)
    wvT_d = nc.declare_dram_parameter("wvT", [DF, J], MDT, False)
    bq_d = nc.declare_dram_parameter("bq", [J], F32, False)
    bk_d = nc.declare_dram_parameter("bk", [J], F32, False)
    woT_d = nc.declare_dram_parameter("woT", [J, DF], MDT, False)
    ones_d = nc.declare_dram_parameter("ones", [128, HEADS * HD], MDT, False)
    out_d = nc.declare_dram_parameter("out", [S, DF], F32, isOutput=True)

    with tile.TileContext(nc) as tc, ExitStack() as ctx:
        persist = ctx.enter_context(tc.tile_pool(name="persist", bufs=1))
        wkvp = ctx.enter_context(tc.tile_pool(name="wkv", bufs=1))
        xs = ctx.enter_context(tc.tile_pool(name="xs", bufs=1))
        ptp = ctx.enter_context(tc.tile_pool(name="pt", bufs=4))
        osbp = ctx.enter_context(tc.tile_pool(name="osb", bufs=4))
        recp = ctx.enter_context(tc.tile_pool(name="rec", bufs=3))
        aux = ctx.enter_context(tc.tile_pool(name="aux", bufs=2, space="PSUM"))
        psS = ctx.enter_context(tc.tile_pool(name="psS", bufs=2, space="PSUM"))
        psPV = ctx.enter_context(tc.tile_pool(name="psPV", bufs=1, space="PSUM"))

        qT = [persist.tile([128, S], MDT, name=f"qT{i}", tag=f"qT{i}") for i in range(4)]
        kT = [persist.tile([128, S], MDT, name=f"kT{i}", tag=f"kT{i}") for i in range(4)]
        vt = [persist.tile([128, HEADS, 2 * HD], MDT, name=f"v{i}", tag=f"v{i}")
              for i in range(16)]
        attnT = [persist.tile([128, S], MDT, name=f"at{i}", tag=f"at{i}")
                 for i in range(4)]
        wq_sb = [persist.tile([128, J], MDT, name=f"wq{k}", tag=f"wq{k}")
                 for k in range(8)]
        wo_sb = [persist.tile([128, DF], MDT, name=f"wo{i}", tag=f"wo{i}")
                 for i in range(4)]
        bq_sb = persist.tile([128, 4], F32, name="bq", tag="bq")
        bk_sb = persist.tile([128, 4], F32, name="bk", tag="bk")


        nc.sync.dma_start(out=bq_sb, in_=bq_d[:].rearrange("(a p) -> p a", p=128))
        nc.sync.dma_start(out=bk_sb, in_=bk_d[:].rearrange("(a p) -> p a", p=128))

        # startup burst: spread the first-chain inputs (wk + x0) over four
        # idle engine DMA rings so the PE can start ~5us in.
        rings = [nc.sync, nc.gpsimd, nc.scalar]
        wk_sb = [wkvp.tile([128, J], MDT, name=f"wk{k}", tag=f"wkv{k}")
                 for k in range(8)]

        def load_x(sc, spread=False):
            ts = []
            for kc in range(8):
                t = xs.tile([128, 512], MDT, name=f"xt{sc % 2}_{kc}",
                            tag=f"xt{sc % 2}_{kc}")
                eng = rings[kc % 3] if spread else nc.sync
                eng.dma_start(
                    out=t,
                    in_=xT_d[128 * kc:128 * (kc + 1), 512 * sc:512 * (sc + 1)])
                ts.append(t)
            return ts

        for k in range(8):
            rings[k % 3].dma_start(out=wk_sb[k], in_=wkT_d[128 * k:128 * (k + 1), :])
        xt_by_sc = {0: load_x(0, spread=True)}
        for k in range(8):
            nc.gpsimd.dma_start(out=wq_sb[k], in_=wqT_d[128 * k:128 * (k + 1), :])
        for i in range(4):
            nc.gpsimd.dma_start(out=wo_sb[i], in_=woT_d[128 * i:128 * (i + 1), :])

        # ---- Phase A: kT (all S), qT (queries 0:1024), then v pass.
        for sc in range(4):
            ss = slice(512 * sc, 512 * (sc + 1))
            xt = xt_by_sc.get(sc) or load_x(sc)
            xt_by_sc[sc] = xt
            for jt in range(4):
                jj = slice(128 * jt, 128 * (jt + 1))
                ps = aux.tile([128, 512], F32, name="aux", tag="aux")
                for kc in range(8):
                    nc.tensor.matmul(ps, wk_sb[kc][:, jj], xt[kc],
                                     start=(kc == 0), stop=(kc == 7))
                nc.vector.tensor_scalar_add(kT[jt][:, ss], ps, bk_sb[:, jt:jt + 1])
                if sc < 2:
                    ps = aux.tile([128, 512], F32, name="aux", tag="aux")
                    for kc in range(8):
                        nc.tensor.matmul(ps, wq_sb[kc][:, jj], xt[kc],
                                         start=(kc == 0), stop=(kc == 7))
                    nc.vector.tensor_scalar_add(qT[jt][:, ss], ps, bq_sb[:, jt:jt + 1])
        # v pass: wv overwrites wk slots (same tags -> WAR-synced)
        wv_sb = [wkvp.tile([128, J], MDT, name=f"wv{k}", tag=f"wkv{k}")
                 for k in range(8)]
        for k in range(8):
            nc.gpsimd.dma_start(out=wv_sb[k], in_=wvT_d[128 * k:128 * (k + 1), :])
        for sc in range(4):
            xt = load_x(sc)
            xt_by_sc[sc] = xt
            for stl in range(4):
                st = 4 * sc + stl
                ps = aux.tile([128, 512], F32, name="aux", tag="aux")
                for kc in range(8):
                    nc.tensor.matmul(ps, xt[kc][:, 128 * stl:128 * (stl + 1)],
                                     wv_sb[kc], start=(kc == 0), stop=(kc == 7))
                nc.vector.tensor_copy(
                    vt[st][:, :, 0:HD], ps[:].rearrange("p (h d) -> p h d", h=HEADS))
                nc.sync.dma_start(
                    out=vt[st][:, :, HD:2 * HD],
                    in_=ones_d[:].rearrange("p (a b) -> p a b", b=HD))

        # ---- Phase B: attention, software-pipelined (pv lags scores by 1 kt),
        # with one filler matmul every other kt:
        #   sc2=0 fillers: deferred q-proj chains (queries 1024:2048), reading
        #     the x chunks still resident in the xs slots from the v pass.
        #   sc2=1 fillers: out-proj of sc2=0's rows, one matmul at a time.
        def qfill_units():
            for sc in (2, 3):
                xt = xt_by_sc[sc]
                ss = slice(512 * sc, 512 * (sc + 1))
                for jt in range(4):
                    jj = slice(128 * jt, 128 * (jt + 1))
                    ps = None
                    for kc in range(8):
                        if kc == 0:
                            ps = aux.tile([128, 512], F32, name="aux", tag="aux")
                        nc.tensor.matmul(ps, wq_sb[kc][:, jj], xt[kc],
                                         start=(kc == 0), stop=(kc == 7))
                        if kc == 7:
                            nc.vector.tensor_scalar_add(
                                qT[jt][:, ss], ps, bq_sb[:, jt:jt + 1])
                        yield

        def emit_oproj_group(st, oc):
            sl = slice(128 * st, 128 * (st + 1))
            ocs = slice(512 * oc, 512 * (oc + 1))
            ps = None
            for jc in range(4):
                if jc == 0:
                    ps = aux.tile([128, 512], F32, name="aux", tag="aux")
                nc.tensor.matmul(ps, attnT[jc][:, sl], wo_sb[jc][:, ocs],
                                 start=(jc == 0), stop=(jc == 3))
                if jc == 3:
                    o_sb = osbp.tile([128, 512], F32, name="osb", tag="osb")
                    nc.vector.tensor_copy(o_sb, ps)
                    nc.gpsimd.dma_start(out=out_d[sl, ocs], in_=o_sb)
                yield

        def oproj_units(sts):
            for st in sts:
                for oc in range(2):
                    yield from emit_oproj_group(st, oc)

        for sc2 in range(2):
            s0 = 1024 * sc2
            filler = qfill_units() if sc2 == 0 else oproj_units(range(8))
            for h in range(HEADS):
                jt, ro = h // 2, 64 * (h % 2)
                rows = slice(ro, ro + 64)
                pv_ps = psPV.tile([128, 1024], F32, name="pv", tag="pv")
                pts = {}

                def emit_pv(k):
                    pt = pts.pop(k)
                    nc.tensor.matmul(pv_ps[:, 0:512], vt[k][:, h, :],
                                     pt[:, 0:512], start=(k == 0), stop=(k == 15))
                    nc.tensor.matmul(pv_ps[:, 512:1024], vt[k][:, h, :],
                                     pt[:, 512:1024], start=(k == 0), stop=(k == 15))

                # PV lags scores by TWO kt: pv(kt-2)'s exp finished a full
                # period ago, so the in-order PE never blocks on the scalar.
                for kt in range(16):
                    tt = slice(128 * kt, 128 * (kt + 1))
                    s_ps = psS.tile([128, 1024], F32, name="sps", tag="sps")
                    nc.tensor.matmul(s_ps[:, 0:512], kT[jt][rows, tt],
                                     qT[jt][rows, s0:s0 + 512])
                    nc.tensor.matmul(s_ps[:, 512:1024], kT[jt][rows, tt],
                                     qT[jt][rows, s0 + 512:s0 + 1024])
                    pt = ptp.tile([128, 1024], MDT, name="pt", tag="pt")
                    nc.scalar.activation(pt, s_ps, mybir.ActivationFunctionType.Exp)
                    pts[kt] = pt
                    if kt >= 2:
                        emit_pv(kt - 2)
                    if kt in (6, 12):
                        next(filler, None)
                emit_pv(14)
                emit_pv(15)
                # rows 64:127 of pv_ps are 64 copies of the softmax sums
                # (V augmented with 64 ones-columns), so 1/Z is a fast
                # 64-partition reciprocal and is ALREADY broadcast; the
                # normalized attnT rows then come from two bank-local PSUM
                # muls. Single-partition DVE ops run ~6ns/elem — avoided
                # entirely. 6 boundary fillers cover the ~3.2us DVE
                # bank-hold on the PE.
                rec64 = recp.tile([64, 1024], MDT, name="rec", tag="rec")
                with nc.allow_low_precision(reason="bf16 1/Z, tol 2e-2"):
                    nc.vector.reciprocal(rec64[:, 0:512], pv_ps[64:128, 0:512])
                    nc.vector.reciprocal(rec64[:, 512:1024],
                                         pv_ps[64:128, 512:1024])
                nc.vector.tensor_mul(attnT[jt][ro:ro + 64, s0:s0 + 512],
                                     pv_ps[0:64, 0:512], rec64[:, 0:512])
                nc.vector.tensor_mul(attnT[jt][ro:ro + 64, s0 + 512:s0 + 1024],
                                     pv_ps[0:64, 512:1024], rec64[:, 512:1024])
                for _ in range(6):
                    next(filler, None)
            for _ in filler:
                pass
        for _ in oproj_units(range(8, 16)):
            pass
    nc.compile()
    return nc


_NC_CACHE = {}


def _get_nc(mm_dt):
    key = str(mm_dt)
    if key not in _NC_CACHE:
        _NC_CACHE[key] = _build(mm_dt)
    return _NC_CACHE[key]


def kernel(**inputs):
    x = np.asarray(inputs["x"], np.float32)
    Wq = np.asarray(inputs["Wq"], np.float32)
    bq = np.asarray(inputs["bq"], np.float32)
    Wk = np.asarray(inputs["Wk"], np.float32)
    bk = np.asarray(inputs["bk"], np.float32)
    Wv = np.asarray(inputs["Wv"], np.float32)
    bv = np.asarray(inputs["bv"], np.float32)
    Wo = np.asarray(inputs["Wo"], np.float32)
    bo = np.asarray(inputs["bo"], np.float32)

    scale = np.float32(1.0 / np.sqrt(HD))
    mm_dt = {"f32r": F32R, "bf16": BF16}[os.environ.get("BASS_MM_DT", "bf16")]
    if mm_dt is BF16:
        import ml_dtypes
        host_dt = ml_dtypes.bfloat16
    else:
        host_dt = np.float32
    nc = _get_nc(mm_dt)

    in_maps = []
    bvwo = []     # host-side bv @ woT rows, one per core
    for c in range(N_CORES):
        b, g = c // 2, c % 2
        cols = slice(J * g, J * (g + 1))
        woTs = np.ascontiguousarray(Wo[:, cols].T)
        in_maps.append({
            "xT": np.ascontiguousarray(x[b].T).astype(host_dt),
            "wqT": (np.ascontiguousarray(Wq[cols, :].T) * scale).astype(host_dt),
            "wkT": np.ascontiguousarray(Wk[cols, :].T).astype(host_dt),
            "wvT": np.ascontiguousarray(Wv[cols, :].T).astype(host_dt),
            "bq": np.ascontiguousarray(bq[cols]) * scale,
            "bk": np.ascontiguousarray(bk[cols]),
            "woT": woTs.astype(host_dt),
            "ones": np.ones((128, HEADS * HD), host_dt),
        })
        bvwo.append(bv[cols] @ woTs)

    res = run_bass_kernel_spmd(
        nc, in_maps, list(range(N_CORES)),
        trace=bool(os.environ.get("BASS_TRACE")))
    LAST_RESULT["exec_time_ns"] = res.exec_time_ns
    LAST_RESULT["mean_exec_time_ns"] = getattr(res, "mean_exec_time_ns", None)
    LAST_RESULT["profile_json"] = res.profile_json
    it = res.instructions_and_trace
    LAST_RESULT["trace_path"] = it[1] if it else None
    LAST_RESULT["insts"] = it[0] if it else None

    B = x.shape[0]
    out = np.empty((B, S, DF), np.float32)
    for b in range(B):
        out[b] = (res.results[2 * b]["out"] + res.results[2 * b + 1]["out"]
                  + bvwo[2 * b][None, :] + bvwo[2 * b + 1][None, :]
                  + bo[None, :])
    return out


# revision 31
# speedup vs baseline: 1.6879x; 1.2311x over previous
"""MHSA Trainium2 kernel: B=4, S=2048, D=1024, H=16 heads of 64.

Sharding (8 cores): core c -> batch b=c//2, head-group g=c%2 (8 heads = 512
proj cols). Identical program on every core; only the data differs.

Per-core layouts (host pre-transposes; bf16 matmul operands, f32 PSUM):
  xT  [1024, 2048] = x[b].T          wqT/wkT/wvT [1024, 512] = W[cols].T
  woT [512, 1024]  = Wo[:, cols].T   out [2048, 1024] partial (host sums pairs)

Device program:
  kT = wkT.T @ xT + bk  (all S); qT = wqT.T @ xT + bq (queries 0:1024 only;
    queries 1024:2048 deferred into the sc2=0 attention loop as PE filler,
    1/8 score scale pre-folded into wqT/bq on host)
  v  = xT.T @ wvT       (v bias deferred: bv @ woT added on host)
  per head h: sT = kT[h].T @ qT[h]; P = exp(sT)  [keys on partitions]
    PV with ones-augmented V: out[65, s] = [V_h | 1].T @ P  -> row 64 = sums
    software-pipelined: PV(kt-1) emitted after scores(kt) so exp overlaps
  per head-pair jt: rrec = 1/sums; partition_broadcast -> rec; attnT *= rec
  out_partial = attnT.T @ woT  (spread as 1-matmul filler units in sc2=1)
"""

import os
from contextlib import ExitStack

import numpy as np

import concourse.bass as bass
import concourse.mybir as mybir


def _install_ntff_shim():
    """The agent image's `antenv` lacks `axon_hooks`, which
    run_bass_kernel_spmd imports when trace=True under axon. Provide it,
    wired to the ctypes NTFF hook from trn_agent_boot when available."""
    import sys
    import types
    try:
        from antenv import axon_hooks  # noqa: F401
        return
    except ImportError:
        pass
    try:
        mod = types.ModuleType("antenv.axon_hooks")
        mod._hook = None
        mod.set_axon_ntff_profile_hook = lambda h: setattr(mod, "_hook", h)
        mod.get_axon_ntff_profile_hook = lambda: mod._hook
        import antenv
        sys.modules["antenv.axon_hooks"] = mod
        antenv.axon_hooks = mod
        try:
            from trn_agent_boot.trn_boot import _ntff_profile_via_ctypes
            import os.path
            so = "/opt/axon/libaxon_pjrt.so"
            if os.path.exists(so):
                mod._hook = _ntff_profile_via_ctypes(so)
        except Exception:
            pass
    except Exception:
        pass


_install_ntff_shim()
import concourse.tile as tile
from concourse import bacc
from concourse.bass_utils import run_bass_kernel_spmd

F32 = mybir.dt.float32
F32R = mybir.dt.float32r
BF16 = mybir.dt.bfloat16

S = 2048       # sequence (rows per core's batch)
DF = 1024      # full model dim (contraction for projections)
J = 512        # proj cols per core (8 heads x 64)
HEADS = 8
HD = 64
N_CORES = 8

LAST_RESULT = {}


def _build(mm_dt):
    MDT = mm_dt
    nc = bacc.Bacc(None, target_bir_lowering=False, debug=False)

    xT_d = nc.declare_dram_parameter("xT", [DF, S], MDT, False)
    wqT_d = nc.declare_dram_parameter("wqT", [DF, J], MDT, False)
    wkT_d = nc.declare_dram_parameter("wkT", [DF, J], MDT, False)
    wvT_d = nc.declare_dram_parameter("wvT", [DF, J], MDT, False)
    bq_d = nc.declare_dram_parameter("bq", [J], F32, False)
    bk_d = nc.declare_dram_parameter("bk", [J], F32, False)
    woT_d = nc.declare_dram_parameter("woT", [J, DF], MDT, False)
    ones_d = nc.declare_dram_parameter("ones", [128, HEADS * HD], MDT, False)
    out_d = nc.declare_dram_parameter("out", [S, DF], F32, isOutput=True)

    with tile.TileContext(nc) as tc, ExitStack() as ctx:
        persist = ctx.enter_context(tc.tile_pool(name="persist", bufs=1))
        wkvp = ctx.enter_context(tc.tile_pool(name="wkv", bufs=1))
        xs = ctx.enter_context(tc.tile_pool(name="xs", bufs=1))
        ptp = ctx.enter_context(tc.tile_pool(name="pt", bufs=4))
        osbp = ctx.enter_context(tc.tile_pool(name="osb", bufs=4))
        recp = ctx.enter_context(tc.tile_pool(name="rec", bufs=3))
        aux = ctx.enter_context(tc.tile_pool(name="aux", bufs=2, space="PSUM"))
        psS = ctx.enter_context(tc.tile_pool(name="psS", bufs=2, space="PSUM"))
        psPV = ctx.enter_context(tc.tile_pool(name="psPV", bufs=1, space="PSUM"))

        qT = [persist.tile([128, S], MDT, name=f"qT{i}", tag=f"qT{i}") for i in range(4)]
        kT = [persist.tile([128, S], MDT, name=f"kT{i}", tag=f"kT{i}") for i in range(4)]
        vt = [persist.tile([128, HEADS, 2 * HD], MDT, name=f"v{i}", tag=f"v{i}")
              for i in range(16)]
        attnT = [persist.tile([128, S], MDT, name=f"at{i}", tag=f"at{i}")
                 for i in range(4)]
        wq_sb = [persist.tile([128, J], MDT, name=f"wq{k}", tag=f"wq{k}")
                 for k in range(8)]
        wo_sb = [persist.tile([128, DF], MDT, name=f"wo{i}", tag=f"wo{i}")
                 for i in range(4)]
        bq_sb = persist.tile([128, 4], F32, name="bq", tag="bq")
        bk_sb = persist.tile([128, 4], F32, name="bk", tag="bk")


        nc.sync.dma_start(out=bq_sb, in_=bq_d[:].rearrange("(a p) -> p a", p=128))
        nc.sync.dma_start(out=bk_sb, in_=bk_d[:].rearrange("(a p) -> p a", p=128))

        # startup burst: spread the first-chain inputs (wk + x0) over four
        # idle engine DMA rings so the PE can start ~5us in.
        rings = [nc.sync, nc.gpsimd, nc.scalar]
        wk_sb = [wkvp.tile([128, J], MDT, name=f"wk{k}", tag=f"wkv{k}")
                 for k in range(8)]

        def load_x(sc, spread=False):
            ts = []
            for kc in range(8):
                t = xs.tile([128, 512], MDT, name=f"xt{sc % 2}_{kc}",
                            tag=f"xt{sc % 2}_{kc}")
                eng = rings[kc % 3] if spread else nc.sync
                eng.dma_start(
                    out=t,
                    in_=xT_d[128 * kc:128 * (kc + 1), 512 * sc:512 * (sc + 1)])
                ts.append(t)
            return ts

        for k in range(8):
            rings[k % 3].dma_start(out=wk_sb[k], in_=wkT_d[128 * k:128 * (k + 1), :])
        xt_by_sc = {0: load_x(0, spread=True)}
        for k in range(8):
            nc.gpsimd.dma_start(out=wq_sb[k], in_=wqT_d[128 * k:128 * (k + 1), :])
        for i in range(4):
            nc.gpsimd.dma_start(out=wo_sb[i], in_=woT_d[128 * i:128 * (i + 1), :])

        # ---- Phase A: kT (all S), qT (queries 0:1024), then v pass.
        for sc in range(4):
            ss = slice(512 * sc, 512 * (sc + 1))
            xt = xt_by_sc.get(sc) or load_x(sc)
            xt_by_sc[sc] = xt
            for jt in range(4):
                jj = slice(128 * jt, 128 * (jt + 1))
                ps = aux.tile([128, 512], F32, name="aux", tag="aux")
                for kc in range(8):
                    nc.tensor.matmul(ps, wk_sb[kc][:, jj], xt[kc],
                                     start=(kc == 0), stop=(kc == 7))
                nc.vector.tensor_scalar_add(kT[jt][:, ss], ps, bk_sb[:, jt:jt + 1])
                if sc < 2:
                    ps = aux.tile([128, 512], F32, name="aux", tag="aux")
                    for kc in range(8):
                        nc.tensor.matmul(ps, wq_sb[kc][:, jj], xt[kc],
                                         start=(kc == 0), stop=(kc == 7))
                    nc.vector.tensor_scalar_add(qT[jt][:, ss], ps, bq_sb[:, jt:jt + 1])
        # v pass: wv overwrites wk slots (same tags -> WAR-synced)
        wv_sb = [wkvp.tile([128, J], MDT, name=f"wv{k}", tag=f"wkv{k}")
                 for k in range(8)]
        for k in range(8):
            nc.gpsimd.dma_start(out=wv_sb[k], in_=wvT_d[128 * k:128 * (k + 1), :])
        for sc in range(4):
            xt = load_x(sc)
            xt_by_sc[sc] = xt
            for stl in range(4):
                st = 4 * sc + stl
                ps = aux.tile([128, 512], F32, name="aux", tag="aux")
                for kc in range(8):
                    nc.tensor.matmul(ps, xt[kc][:, 128 * stl:128 * (stl + 1)],
                                     wv_sb[kc], start=(kc == 0), stop=(kc == 7))
                nc.vector.tensor_copy(
                    vt[st][:, :, 0:HD], ps[:].rearrange("p (h d) -> p h d", h=HEADS))
                nc.sync.dma_start(
                    out=vt[st][:, :, HD:2 * HD],
                    in_=ones_d[:].rearrange("p (a b) -> p a b", b=HD))

        # ---- Phase B: attention, software-pipelined (pv lags scores by 1 kt),
        # with one filler matmul every other kt:
        #   sc2=0 fillers: deferred q-proj chains (queries 1024:2048), reading
        #     the x chunks still resident in the xs slots from the v pass.
        #   sc2=1 fillers: out-proj of sc2=0's rows, one matmul at a time.
        def qfill_units():
            for sc in (2, 3):
                xt = xt_by_sc[sc]
                ss = slice(512 * sc, 512 * (sc + 1))
                for jt in range(4):
                    jj = slice(128 * jt, 128 * (jt + 1))
                    ps = None
                    for kc in range(8):
                        if kc == 0:
                            ps = aux.tile([128, 512], F32, name="aux", tag="aux")
                        nc.tensor.matmul(ps, wq_sb[kc][:, jj], xt[kc],
                                         start=(kc == 0), stop=(kc == 7))
                        if kc == 7:
                            nc.vector.tensor_scalar_add(
                                qT[jt][:, ss], ps, bq_sb[:, jt:jt + 1])
                        yield

        def emit_oproj_group(st, oc):
            sl = slice(128 * st, 128 * (st + 1))
            ocs = slice(512 * oc, 512 * (oc + 1))
            ps = None
            for jc in range(4):
                if jc == 0:
                    ps = aux.tile([128, 512], F32, name="aux", tag="aux")
                nc.tensor.matmul(ps, attnT[jc][:, sl], wo_sb[jc][:, ocs],
                                 start=(jc == 0), stop=(jc == 3))
                if jc == 3:
                    o_sb = osbp.tile([128, 512], F32, name="osb", tag="osb")
                    nc.vector.tensor_copy(o_sb, ps)
                    nc.gpsimd.dma_start(out=out_d[sl, ocs], in_=o_sb)
                yield

        def oproj_units(sts):
            for st in sts:
                for oc in range(2):
                    yield from emit_oproj_group(st, oc)

        for sc2 in range(2):
            s0 = 1024 * sc2
            filler = qfill_units() if sc2 == 0 else oproj_units(range(8))
            for h in range(HEADS):
                jt, ro = h // 2, 64 * (h % 2)
                rows = slice(ro, ro + 64)
                pv_ps = psPV.tile([128, 1024], F32, name="pv", tag="pv")
                pts = {}

                def emit_pv(k):
                    pt = pts.pop(k)
                    nc.tensor.matmul(pv_ps[:, 0:512], vt[k][:, h, :],
                                     pt[:, 0:512], start=(k == 0), stop=(k == 15))
                    nc.tensor.matmul(pv_ps[:, 512:1024], vt[k][:, h, :],
                                     pt[:, 512:1024], start=(k == 0), stop=(k == 15))

                # PV lags scores by TWO kt: pv(kt-2)'s exp finished a full
                # period ago, so the in-order PE never blocks on the scalar.
                for kt in range(16):
                    tt = slice(128 * kt, 128 * (kt + 1))
                    s_ps = psS.tile([128, 1024], F32, name="sps", tag="sps")
                    nc.tensor.matmul(s_ps[:, 0:512], kT[jt][rows, tt],
                                     qT[jt][rows, s0:s0 + 512])
                    nc.tensor.matmul(s_ps[:, 512:1024], kT[jt][rows, tt],
                                     qT[jt][rows, s0 + 512:s0 + 1024])
                    pt = ptp.tile([128, 1024], MDT, name="pt", tag="pt")
                    nc.scalar.activation(pt, s_ps, mybir.ActivationFunctionType.Exp)
                    pts[kt] = pt
                    if kt >= 2:
                        emit_pv(kt - 2)
                    if kt in (6, 12):
                        next(filler, None)
                emit_pv(14)
                emit_pv(15)
                # rows 64:127 of pv_ps are 64 copies of the softmax sums
                # (V augmented with 64 ones-columns), so 1/Z is a fast
                # 64-partition reciprocal and is ALREADY broadcast; the
                # normalized attnT rows then come from two bank-local PSUM
                # muls. Single-partition DVE ops run ~6ns/elem — avoided
                # entirely. 6 boundary fillers cover the ~3.2us DVE
                # bank-hold on the PE.
                # the approx reciprocal (custom DVE op) reads Z from an
                # offset-0 SBUF staging tile: fed straight from PSUM at
                # partition offset 64 it returns deterministic garbage, and
                # the tracked copies double as the dep fence for it.
                zb = recp.tile([64, 1024], F32, name="zb", tag="zb")
                nc.vector.tensor_copy(zb[:, 0:512], pv_ps[64:128, 0:512])
                nc.vector.tensor_copy(zb[:, 512:1024], pv_ps[64:128, 512:1024])
                rec64 = recp.tile([64, 1024], F32, name="rec", tag="rec")
                nc.vector.reciprocal_approx_fast(rec64[:, 0:512], zb[:, 0:512])
                nc.vector.reciprocal_approx_fast(rec64[:, 512:1024],
                                                 zb[:, 512:1024])
                nc.vector.tensor_mul(attnT[jt][ro:ro + 64, s0:s0 + 512],
                                     pv_ps[0:64, 0:512], rec64[:, 0:512])
                nc.vector.tensor_mul(attnT[jt][ro:ro + 64, s0 + 512:s0 + 1024],
                                     pv_ps[0:64, 512:1024], rec64[:, 512:1024])
                for _ in range(6):
                    next(filler, None)
            for _ in filler:
                pass
        for _ in oproj_units(range(8, 16)):
            pass
    nc.compile()
    return nc


_NC_CACHE = {}


def _get_nc(mm_dt):
    key = str(mm_dt)
    if key not in _NC_CACHE:
        _NC_CACHE[key] = _build(mm_dt)
    return _NC_CACHE[key]


def kernel(**inputs):
    x = np.asarray(inputs["x"], np.float32)
    Wq = np.asarray(inputs["Wq"], np.float32)
    bq = np.asarray(inputs["bq"], np.float32)
    Wk = np.asarray(inputs["Wk"], np.float32)
    bk = np.asarray(inputs["bk"], np.float32)
    Wv = np.asarray(inputs["Wv"], np.float32)
    bv = np.asarray(inputs["bv"], np.float32)
    Wo = np.asarray(inputs["Wo"], np.float32)
    bo = np.asarray(inputs["bo"], np.float32)

    scale = np.float32(1.0 / np.sqrt(HD))
    mm_dt = {"f32r": F32R, "bf16": BF16}[os.environ.get("BASS_MM_DT", "bf16")]
    if mm_dt is BF16:
        import ml_dtypes
        host_dt = ml_dtypes.bfloat16
    else:
        host_dt = np.float32
    nc = _get_nc(mm_dt)

    in_maps = []
    bvwo = []     # host-side bv @ woT rows, one per core
    for c in range(N_CORES):
        b, g = c // 2, c % 2
        cols = slice(J * g, J * (g + 1))
        woTs = np.ascontiguousarray(Wo[:, cols].T)
        in_maps.append({
            "xT": np.ascontiguousarray(x[b].T).astype(host_dt),
            "wqT": (np.ascontiguousarray(Wq[cols, :].T) * scale).astype(host_dt),
            "wkT": np.ascontiguousarray(Wk[cols, :].T).astype(host_dt),
            "wvT": np.ascontiguousarray(Wv[cols, :].T).astype(host_dt),
            "bq": np.ascontiguousarray(bq[cols]) * scale,
            "bk": np.ascontiguousarray(bk[cols]),
            "woT": woTs.astype(host_dt),
            "ones": np.ones((128, HEADS * HD), host_dt),
        })
        bvwo.append(bv[cols] @ woTs)

    res = run_bass_kernel_spmd(
        nc, in_maps, list(range(N_CORES)),
        trace=bool(os.environ.get("BASS_TRACE")))
    LAST_RESULT["exec_time_ns"] = res.exec_time_ns
    LAST_RESULT["mean_exec_time_ns"] = getattr(res, "mean_exec_time_ns", None)
    LAST_RESULT["profile_json"] = res.profile_json
    it = res.instructions_and_trace
    LAST_RESULT["trace_path"] = it[1] if it else None
    LAST_RESULT["insts"] = it[0] if it else None

    B = x.shape[0]
    out = np.empty((B, S, DF), np.float32)
    for b in range(B):
        out[b] = (res.results[2 * b]["out"] + res.results[2 * b + 1]["out"]
                  + bvwo[2 * b][None, :] + bvwo[2 * b + 1][None, :]
                  + bo[None, :])
    return out
